# revision 16
# baseline (speedup 1.0000x reference)
"""Fused multi-head attention (QKV proj + RoPE + causal softmax + out proj)
for Trainium2, sharded over 8 NeuronCores.

Sharding: data-parallel over batch (B=2) x tensor-parallel over heads
(16 heads -> 4 per core).  Each core computes, for its (batch, head-group):
  qT/kT = wq/wk^T-projections in [d, s] layout (CDT matmuls, fp32 PSUM)
  RoPE applied on-chip (DVE pair-swap via stream_shuffle + mul/add)
  scoresT[kp, q] = krot^T.T @ qrot (one K=128 matmul per tile)
  causal masking via a PE-accumulated triangular constant on diagonal tiles
  expT = exp(scale * scoresT) on ACT
  transposed PV: attnT[d, q] += vaug[k, d].T @ expT[k, q]  (512-wide matmuls)
  denominator: DVE-accumulated exp sums -> one all-ones f32r matmul that both
  sums across partitions and broadcasts the result; reciprocal folded into
  the attnT drain (PSUM * rec -> SBUF)
  partial output y_g = attnT.T @ wo_rows  (summed over head-groups on host),
  software-pipelined into the next q-block's score/PV loops so the PE never
  waits on ACT exp latency.

Weights are loaded once into resident SBUF tiles (host pre-arranges them
into the on-chip [P, chunk, free] layout so DMA descriptors are large);
x arrives as per-s-block panels reused by all three projections.

Inputs arrive full-size; host slices/transposes, feeds 8 SPMD cores, and
sums the 4 head-group partials per batch at the end.
"""

import math

import numpy as np

import concourse.bacc as bacc
import concourse.mybir as mybir
from concourse import tile
from concourse.bass_utils import run_bass_kernel_spmd

B, S, D, H = 2, 2048, 2048, 16
NCORES = 8
HG = 4  # heads per core
HD = D // H  # 128
DG = HG * HD  # 512 = per-core slice of D
P = 128
NKC = D // P  # 16 contraction chunks
SBLK = 512  # s-block width in projection passes
NSB = S // SBLK
NST = S // P  # 16 s-tiles of 128
QB = 512  # q-block width in attention
NQB = S // QB
NQT = QB // P  # q-subtiles per block
EB = 512  # e-block width in out-projection
NEB = D // EB

F32 = mybir.dt.float32
F32R = mybir.dt.float32r
EXP = mybir.ActivationFunctionType.Exp
SCALE = 1.0 / math.sqrt(HD)
SWAP32 = [i ^ 1 for i in range(32)]
NEG = -1.0e9

COMPUTE_DTYPE = "bfloat16"


def build_program(variant: str, dump: bool = False, cdt_name: str | None = None):
    """variant: 'causal' | 'none' | 'general'"""
    CDT = getattr(mybir.dt, cdt_name or COMPUTE_DTYPE)
    nc = bacc.Bacc("TRN2", target_bir_lowering=False, debug=False)
    # host pre-arranged layouts (see make_in_maps):
    #   xT  [P, NSB, NKC, SBLK]   x^T chunked: [p, sb, kc, s'] = x[sb*512+s', kc*128+p]
    #   wq/wk/wv [P, NKC, DG]     [p, kc, d'] = w[kc*128+p, d']
    #   wo  [P, HG, D]            [p, dc, e]  = wo[dc*128+p, e]
    xT = nc.dram_tensor("xT", [P, NSB, NKC, SBLK], CDT, kind="ExternalInput")
    wq = nc.dram_tensor("wq", [P, NKC, DG], CDT, kind="ExternalInput")
    wk = nc.dram_tensor("wk", [P, NKC, DG], CDT, kind="ExternalInput")
    wv = nc.dram_tensor("wv", [P, NKC, DG], CDT, kind="ExternalInput")
    wo = nc.dram_tensor("wo", [P, HG, D], CDT, kind="ExternalInput")
    cosT = nc.dram_tensor("cosT", [HD, S], CDT, kind="ExternalInput")
    sinT = nc.dram_tensor("sinT", [HD, S], CDT, kind="ExternalInput")
    ident = nc.dram_tensor("ident", [P, P], CDT, kind="ExternalInput")
    tri = None
    maskT = None
    if variant == "causal":
        tri = nc.dram_tensor("tri", [P, P], CDT, kind="ExternalInput")
    elif variant == "general":
        # mask.T pre-scaled by sqrt(HD) on host so exp's scale recovers it
        maskT = nc.dram_tensor("maskT", [S, S], CDT, kind="ExternalInput")
    y = nc.dram_tensor("y", [S, D], F32, kind="ExternalOutput")
    d_qrot = d_krot = d_vaug = d_attnT = None
    if dump:
        d_qrot = nc.dram_tensor("d_qrot", [P, HG, S], CDT, kind="ExternalOutput")
        d_krot = nc.dram_tensor("d_krot", [P, HG, S], CDT, kind="ExternalOutput")
        d_vaug = nc.dram_tensor("d_vaug", [P, NST, HG, HD + 2], CDT, kind="ExternalOutput")
        d_attnT = nc.dram_tensor("d_attnT", [P, HG, S], CDT, kind="ExternalOutput")

    with tile.TileContext(nc) as tc:
        with (
            tc.tile_pool(name="const", bufs=1) as constp,
            tc.tile_pool(name="big", bufs=1) as bigp,
            # one PSUM pool shared by every phase: no pool-scoping barriers,
            # so attention matmuls can start while the v-pass drains.
            tc.tile_pool(name="ps", bufs=6, space="PSUM") as pspool,
            tc.tile_pool(name="psaux", bufs=2, space="PSUM") as psaux,
        ):
            qrot = bigp.tile([P, HG, S], CDT, tag="qrot")
            krot = bigp.tile([P, HG, S], CDT, tag="krot")
            vaug = bigp.tile([P, NST, HG, HD + 2], CDT, tag="vaug")

            # ---------------- projections + RoPE ----------------
            # Weights are resident (loaded once; the first chunks are split
            # across partitions+engines so the first matmul starts ~3us in);
            # x panels stream per s-block and are reused by all three
            # projections.
            with (
                tc.tile_pool(name="wres", bufs=1) as wres,
                tc.tile_pool(name="xpool", bufs=2) as xpool,
                tc.tile_pool(name="rope", bufs=3) as ropep,
            ):
                wq_r = wres.tile([P, NKC, DG], CDT, tag="wq")
                wk_r = wres.tile([P, NKC, DG], CDT, tag="wk")
                wv_r = wres.tile([P, NKC, DG], CDT, tag="wv")
                xt0 = xpool.tile([P, NKC, SBLK], CDT, tag="xt", name="xp0")
                ENG = [nc.sync, nc.scalar, nc.gpsimd, nc.sync]
                # chunk 0 of wq and x-panel 0: 32-partition slices on four
                # engines each -> ~2us to first matmul
                for i, e in enumerate(ENG):
                    psl = slice(i * 32, (i + 1) * 32)
                    e.dma_start(wq_r[psl, 0, :], wq[psl, 0, :])
                for i, e in enumerate(ENG):
                    psl = slice(i * 32, (i + 1) * 32)
                    e.dma_start(xt0[psl, 0, :], xT[psl, 0, 0, :])
                # chunks 1-3 split in partition halves on rotating engines
                for c in range(1, 4):
                    for i2 in range(2):
                        psl = slice(i2 * 64, (i2 + 1) * 64)
                        ENG[(2 * c + i2) % 3].dma_start(
                            wq_r[psl, c, :], wq[psl, c, :]
                        )
                        ENG[(2 * c + i2 + 1) % 3].dma_start(
                            xt0[psl, c, :], xT[psl, 0, c, :]
                        )
                tid = constp.tile([P, P], CDT)
                nc.scalar.dma_start(tid[:], ident[:])
                ttri = None
                if variant == "causal":
                    ttri = constp.tile([P, P], CDT)
                    nc.scalar.dma_start(ttri[:], tri[:])
                tcos = constp.tile([HD, S], CDT)
                nc.scalar.dma_start(tcos[:], cosT[:])
                tsin = constp.tile([HD, S], CDT)
                nc.scalar.dma_start(tsin[:], sinT[:])
                for c2 in range(2, 8):
                    sl = slice(c2 * 2, c2 * 2 + 2)
                    ENG[c2 % 3].dma_start(wq_r[:, sl, :], wq[:, sl, :])
                    ENG[(c2 + 1) % 3].dma_start(xt0[:, sl, :], xT[:, 0, sl, :])
                for g in range(4):
                    nc.sync.dma_start(wk_r[:, g * 4 : g * 4 + 4, :], wk[:, g * 4 : g * 4 + 4, :])
                for g in range(4):
                    nc.sync.dma_start(wv_r[:, g * 4 : g * 4 + 4, :], wv[:, g * 4 : g * 4 + 4, :])
                wmap = {"q": wq_r, "k": wk_r, "v": wv_r}

                ones128 = constp.tile([P, P], CDT)
                nc.vector.memset(ones128, 1.0)

                for sb in range(NSB):
                    if sb == 0:
                        xt = xt0
                    else:
                        xt = xpool.tile([P, NKC, SBLK], CDT, tag="xt", name=f"xp{sb}")
                        for g in range(4):
                            nc.gpsimd.dma_start(
                                xt[:, g * 4 : g * 4 + 4, :], xT[:, sb, g * 4 : g * 4 + 4, :]
                            )
                    for proj in ("q", "k", "v"):
                        w_r = wmap[proj]
                        nun = SBLK // P if proj == "v" else HG
                        pss = [
                            pspool.tile(
                                [P, SBLK if proj != "v" else DG],
                                F32,
                                tag="ps",
                                name=f"ps_{proj}_{sb}_{u}",
                            )
                            for u in range(nun)
                        ]
                        for kc in range(NKC):
                            if proj in ("q", "k"):
                                for dt in range(HG):
                                    nc.tensor.matmul(
                                        pss[dt][:],
                                        w_r[:, kc, dt * HD : (dt + 1) * HD],
                                        xt[:, kc, :],
                                        start=(kc == 0),
                                        stop=(kc == NKC - 1),
                                    )
                            else:
                                for st in range(SBLK // P):
                                    nc.tensor.matmul(
                                        pss[st][:],
                                        xt[:, kc, st * P : (st + 1) * P],
                                        w_r[:, kc, :],
                                        start=(kc == 0),
                                        stop=(kc == NKC - 1),
                                    )
                        if proj in ("q", "k"):
                            dstbuf = qrot if proj == "q" else krot
                            ssl = slice(sb * SBLK, (sb + 1) * SBLK)
                            # drain all four PSUM banks first (fast copies),
                            # then run the RoPE chains from SBUF
                            qsbs = []
                            for dt in range(HG):
                                qsb = ropep.tile(
                                    [P, SBLK], CDT, tag=f"qsb{dt}", name="qsb"
                                )
                                nc.vector.tensor_copy(qsb[:], pss[dt][:])
                                qsbs.append(qsb)
                            for dt in range(HG):
                                qsb = qsbs[dt]
                                tsw = ropep.tile([P, SBLK], CDT, tag="tsw", name="tsw")
                                nc.vector.stream_shuffle(tsw[:], qsb[:], SWAP32)
                                t1 = ropep.tile([P, SBLK], CDT, tag="t1", name="t1")
                                nc.vector.tensor_mul(t1[:], qsb[:], tcos[:, ssl])
                                t2 = ropep.tile([P, SBLK], CDT, tag="t2", name="t2")
                                nc.vector.tensor_mul(t2[:], tsw[:], tsin[:, ssl])
                                nc.vector.tensor_add(
                                    dstbuf[:, dt, ssl], t1[:], t2[:]
                                )
                        else:
                            for st in range(SBLK // P):
                                st_g = sb * (SBLK // P) + st
                                nc.vector.tensor_copy(
                                    vaug[:, st_g, :, 0:HD],
                                    pss[st][:].rearrange("p (h d) -> p h d", d=HD),
                                )

            if dump:
                nc.sync.dma_start(d_qrot.ap(), qrot[:])
                nc.sync.dma_start(d_krot.ap(), krot[:])
                nc.sync.dma_start(d_vaug.ap(), vaug[:])

            # ---------------- attention (+ wo weights prefetch) ----------------
            with (
                tc.tile_pool(name="attn_out", bufs=1) as atp,
                tc.tile_pool(name="wopool", bufs=1) as wopool,
            ):
              attnT = atp.tile([P, HG, S], CDT, tag="attnT")
              wo_sb = wopool.tile([P, HG, D], CDT, tag="wo")
              nc.sync.dma_start(wo_sb[:], wo[:])
              with (
                tc.tile_pool(name="mask", bufs=2) as maskp,
                tc.tile_pool(name="expp", bufs=6) as epool,
                tc.tile_pool(name="esum", bufs=2) as esump,
                tc.tile_pool(name="normp", bufs=2) as npool,
                tc.tile_pool(name="outp", bufs=3) as outp,
              ):
                # out-projection work items (st, eb) deferred from the
                # previous q-block; emitted between score/PV matmuls so the
                # PE stays busy while ACT computes the exps.
                pending: list[tuple[int, int]] = []
                dma_eng = [nc.sync, nc.scalar, nc.gpsimd]
                dma_ctr = [0]

                def emit_outproj(n: int):
                    for _ in range(n):
                        if not pending:
                            return
                        st, eb = pending.pop(0)
                        ps_o = psaux.tile([P, EB], F32, tag="tr", name=f"o{st}_{eb}")
                        for dc in range(HG):
                            nc.tensor.matmul(
                                ps_o[:],
                                attnT[:, dc, st * P : (st + 1) * P],
                                wo_sb[:, dc, eb * EB : (eb + 1) * EB],
                                start=(dc == 0),
                                stop=(dc == HG - 1),
                            )
                        out_t = outp.tile([P, EB], F32, tag="outsb", name="outsb")
                        nc.vector.tensor_copy(out_t[:], ps_o[:])
                        eng = dma_eng[dma_ctr[0] % 3]
                        dma_ctr[0] += 1
                        eng.dma_start(
                            y[st * P : (st + 1) * P, eb * EB : (eb + 1) * EB],
                            out_t[:],
                        )

                # deferred per-head normalization: the denominator matmul +
                # reciprocal + attnT drain for head h are emitted early in
                # head h+1's score loop, hiding the DVE esum-chain latency.
                def emit_den(state):
                    att_ps, esum, s_qb, s_h = state
                    esum_bf = npool.tile([P, QB], CDT, tag="ebf", name="ebf")
                    nc.vector.tensor_copy(esum_bf[:], esum[:])
                    # sum across partitions AND broadcast, in one all-ones
                    # bf16 matmul (fp32r would downclock the whole PE)
                    den_ps = psaux.tile([P, QB], F32, tag="tr", name="den")
                    nc.tensor.matmul(
                        den_ps[:], ones128[:], esum_bf[:], start=True, stop=True
                    )
                    rec = npool.tile([P, QB], F32, tag="rec", name="rec")
                    nc.vector.reciprocal_approx_fast(rec[:], den_ps[:])
                    nc.vector.tensor_mul(
                        attnT[:, s_h, s_qb * QB : (s_qb + 1) * QB],
                        att_ps[:],
                        rec[:],
                    )

                prev_state = None
                for qb in range(NQB):
                    mts = None
                    if variant == "general":
                        mts = maskp.tile([P, NST, QB], CDT, tag="mt", name="mt")
                        nc.sync.dma_start(
                            mts[:],
                            maskT[:, qb * QB : (qb + 1) * QB].rearrange(
                                "(kt p) q -> p kt q", p=P
                            ),
                        )
                    nkt = NQT * (qb + 1) if variant == "causal" else NST
                    for h in range(HG):
                        att_ps = pspool.tile(
                            [P, QB], F32, tag="ps", name=f"att_{qb}_{h}"
                        )
                        esum = esump.tile([P, QB], F32, tag="esum", name="esum")
                        texp0 = None
                        den_at = 0 if h == 0 else 1
                        for kt in range(nkt):
                            j = kt - NQT * qb  # diag index (causal)
                            diag = variant == "causal" and j >= 0
                            ps_s = pspool.tile([P, QB], F32, tag="ps", name="scores")
                            if diag:
                                nc.tensor.matmul(
                                    ps_s[:, j * P : QB],
                                    krot[:, h, kt * P : (kt + 1) * P],
                                    qrot[:, h, qb * QB + j * P : (qb + 1) * QB],
                                    start=True,
                                    stop=False,
                                )
                                nc.tensor.matmul(
                                    ps_s[:, j * P : (j + 1) * P],
                                    tid[:],
                                    ttri[:],
                                    start=False,
                                    stop=True,
                                )
                                valid = slice(j * P, QB)
                            else:
                                last = variant != "general"
                                nc.tensor.matmul(
                                    ps_s[:],
                                    krot[:, h, kt * P : (kt + 1) * P],
                                    qrot[:, h, qb * QB : (qb + 1) * QB],
                                    start=True,
                                    stop=last,
                                )
                                if variant == "general":
                                    nc.tensor.matmul(
                                        ps_s[:],
                                        tid[:],
                                        mts[:, kt, :],
                                        start=False,
                                        stop=True,
                                    )
                                valid = slice(0, QB)
                            texp = epool.tile([P, QB], CDT, tag="exp", name="exp")
                            nc.scalar.activation(
                                texp[:, valid], ps_s[:, valid], EXP, scale=SCALE
                            )
                            # transposed PV accumulate: attnT_ps[d, q]
                            nc.tensor.matmul(
                                att_ps[:, valid],
                                vaug[:, kt, h, 0:HD],
                                texp[:, valid],
                                start=(kt == 0),
                                stop=(kt == nkt - 1),
                                skip_group_check=(variant == "causal"),
                            )
                            # denominator partials on DVE (esum starts at the
                            # kt==1 pair so the chain is one op shorter)
                            if kt == 0:
                                texp0 = texp
                            elif kt == 1 and not diag:
                                nc.vector.tensor_add(
                                    esum[:], texp0[:], texp[:]
                                )
                            else:
                                if kt == 1:
                                    # qb==0: kt1 is already diagonal/partial
                                    nc.vector.tensor_copy(esum[:], texp0[:])
                                nc.vector.tensor_add(
                                    esum[:, valid],
                                    esum[:, valid],
                                    texp[:, valid],
                                )
                            if kt == den_at and prev_state is not None:
                                emit_den(prev_state)
                                prev_state = None
                            # spread the deferred out-proj fill work evenly
                            # over this q-block's score/PV slots (16 items
                            # from qb-1 across 16*(qb+1) kt slots)
                            if kt % (qb + 1) == qb:
                                emit_outproj(1)
                        prev_state = (att_ps, esum, qb, h)
                    pending.extend(
                        (st, eb)
                        for st in range(qb * NQT, qb * NQT + NQT)
                        for eb in range(NEB)
                    )
                # final head's normalization + the last q-block's out-proj
                emit_den(prev_state)
                emit_outproj(len(pending))

              if dump:
                  nc.sync.dma_start(d_attnT.ap(), attnT[:])

    nc.compile()
    return nc


_PROGRAM_CACHE: dict[str, object] = {}
_last_in_maps = None


def _get_program(variant: str):
    key = f"{variant}:{COMPUTE_DTYPE}"
    if key not in _PROGRAM_CACHE:
        _PROGRAM_CACHE[key] = build_program(variant)
    return _PROGRAM_CACHE[key]


def _detect_variant(mask: np.ndarray) -> str:
    if not np.any(mask):
        return "none"
    causal = np.triu(np.full((S, S), NEG, dtype=np.float32), 1)
    if np.array_equal(mask, causal):
        return "causal"
    return "general"


def _np_cdt():
    if COMPUTE_DTYPE == "bfloat16":
        import ml_dtypes

        return ml_dtypes.bfloat16
    return np.float32


def make_in_maps(x, wq, wk, wv, wo, cos, sin, mask, variant):
    npdt = _np_cdt()
    cosT = np.repeat(cos.T, 2, axis=0)  # [HD, S]
    sinT = np.repeat(sin.T, 2, axis=0)
    sinT = sinT.copy()
    sinT[0::2, :] *= -1.0  # row 2i holds -sin, row 2i+1 holds +sin
    shared = {
        "cosT": np.ascontiguousarray(cosT).astype(npdt),
        "sinT": np.ascontiguousarray(sinT).astype(npdt),
        "ident": np.eye(P, dtype=np.float32).astype(npdt),
    }
    if variant == "causal":
        # scoresT layout is [kp, q]: masked where kp > q -> strict lower triangle
        shared["tri"] = np.tril(np.full((P, P), NEG, dtype=np.float32), -1).astype(npdt)
    elif variant == "general":
        shared["maskT"] = np.ascontiguousarray(mask.T * math.sqrt(HD)).astype(npdt)

    # xT chunked to [P, NSB, NKC, SBLK]: [p, sb, kc, s'] = x[b, sb*SBLK+s', kc*P+p]
    xTs = [
        np.ascontiguousarray(
            x[b].T.reshape(NKC, P, NSB, SBLK).transpose(1, 2, 0, 3)
        ).astype(npdt)
        for b in range(B)
    ]
    in_maps = []
    for core in range(NCORES):
        b, g = divmod(core, NCORES // B)
        sl = slice(g * DG, (g + 1) * DG)
        in_maps.append(
            {
                "xT": xTs[b],
                "wq": np.ascontiguousarray(
                    wq[:, sl].reshape(NKC, P, DG).transpose(1, 0, 2)
                ).astype(npdt),
                "wk": np.ascontiguousarray(
                    wk[:, sl].reshape(NKC, P, DG).transpose(1, 0, 2)
                ).astype(npdt),
                "wv": np.ascontiguousarray(
                    wv[:, sl].reshape(NKC, P, DG).transpose(1, 0, 2)
                ).astype(npdt),
                "wo": np.ascontiguousarray(
                    wo[sl, :].reshape(HG, P, D).transpose(1, 0, 2)
                ).astype(npdt),
                **shared,
            }
        )
    return in_maps


def kernel(x, wq, wk, wv, wo, cos, sin, mask):
    x = np.asarray(x, dtype=np.float32)
    wq = np.asarray(wq, dtype=np.float32)
    wk = np.asarray(wk, dtype=np.float32)
    wv = np.asarray(wv, dtype=np.float32)
    wo = np.asarray(wo, dtype=np.float32)
    cos = np.asarray(cos, dtype=np.float32)
    sin = np.asarray(sin, dtype=np.float32)
    mask = np.asarray(mask, dtype=np.float32)

    variant = _detect_variant(mask)
    nc = _get_program(variant)
    in_maps = make_in_maps(x, wq, wk, wv, wo, cos, sin, mask, variant)

    global _last_in_maps
    _last_in_maps = in_maps

    res = run_bass_kernel_spmd(nc, in_maps, core_ids=list(range(NCORES)))

    out = np.empty((B, S, D), dtype=np.float32)
    gpb = NCORES // B
    for b in range(B):
        acc = np.zeros((S, D), dtype=np.float64)
        for g in range(gpb):
            acc += res.results[b * gpb + g]["y"].astype(np.float64)
        out[b] = acc.astype(np.float32)
    return out


# revision 22
# speedup vs baseline: 1.0114x; 1.0114x over previous
"""Fused multi-head attention (QKV proj + RoPE + causal softmax + out proj)
for Trainium2, sharded over 8 NeuronCores.

Sharding: data-parallel over batch (B=2) x tensor-parallel over heads
(16 heads -> 4 per core).  Each core computes, for its (batch, head-group):
  qT/kT = wq/wk^T-projections in [d, s] layout (CDT matmuls, fp32 PSUM)
  RoPE applied on-chip (DVE pair-swap via stream_shuffle + mul/add)
  scoresT[kp, q] = krot^T.T @ qrot (one K=128 matmul per tile)
  causal masking via a PE-accumulated triangular constant on diagonal tiles
  expT = exp(scale * scoresT) on ACT
  transposed PV: attnT[d, q] += vaug[k, d].T @ expT[k, q]  (512-wide matmuls)
  denominator: DVE-accumulated exp sums -> one all-ones f32r matmul that both
  sums across partitions and broadcasts the result; reciprocal folded into
  the attnT drain (PSUM * rec -> SBUF)
  partial output y_g = attnT.T @ wo_rows  (summed over head-groups on host),
  software-pipelined into the next q-block's score/PV loops so the PE never
  waits on ACT exp latency.

Weights are loaded once into resident SBUF tiles (host pre-arranges them
into the on-chip [P, chunk, free] layout so DMA descriptors are large);
x arrives as per-s-block panels reused by all three projections.

Inputs arrive full-size; host slices/transposes, feeds 8 SPMD cores, and
sums the 4 head-group partials per batch at the end.
"""

import math

import numpy as np

import concourse.bacc as bacc
import concourse.mybir as mybir
from concourse import tile
from concourse.bass_utils import run_bass_kernel_spmd

B, S, D, H = 2, 2048, 2048, 16
NCORES = 8
HG = 4  # heads per core
HD = D // H  # 128
DG = HG * HD  # 512 = per-core slice of D
P = 128
NKC = D // P  # 16 contraction chunks
SBLK = 512  # s-block width in projection passes
NSB = S // SBLK
NST = S // P  # 16 s-tiles of 128
QB = 512  # q-block width in attention
NQB = S // QB
NQT = QB // P  # q-subtiles per block
EB = 512  # e-block width in out-projection
NEB = D // EB

F32 = mybir.dt.float32
F32R = mybir.dt.float32r
EXP = mybir.ActivationFunctionType.Exp
SCALE = 1.0 / math.sqrt(HD)
SWAP32 = [i ^ 1 for i in range(32)]
NEG = -1.0e9

COMPUTE_DTYPE = "bfloat16"


def build_program(variant: str, dump: bool = False, cdt_name: str | None = None):
    """variant: 'causal' | 'none' | 'general'"""
    CDT = getattr(mybir.dt, cdt_name or COMPUTE_DTYPE)
    nc = bacc.Bacc("TRN2", target_bir_lowering=False, debug=False)
    # host pre-arranged layouts (see make_in_maps):
    #   xT  [P, NSB, NKC, SBLK]   x^T chunked: [p, sb, kc, s'] = x[sb*512+s', kc*128+p]
    #   wq/wk/wv [P, NKC, DG]     [p, kc, d'] = w[kc*128+p, d']
    #   wo  [P, HG, D]            [p, dc, e]  = wo[dc*128+p, e]
    xT = nc.dram_tensor("xT", [P, NSB, NKC, SBLK], CDT, kind="ExternalInput")
    wq = nc.dram_tensor("wq", [P, NKC, DG], CDT, kind="ExternalInput")
    wk = nc.dram_tensor("wk", [P, NKC, DG], CDT, kind="ExternalInput")
    wv = nc.dram_tensor("wv", [P, NKC, DG], CDT, kind="ExternalInput")
    wo = nc.dram_tensor("wo", [P, HG, D], CDT, kind="ExternalInput")
    cosT = nc.dram_tensor("cosT", [HD, S], CDT, kind="ExternalInput")
    sinT = nc.dram_tensor("sinT", [HD, S], CDT, kind="ExternalInput")
    ident = nc.dram_tensor("ident", [P, P], CDT, kind="ExternalInput")
    tri = None
    maskT = None
    if variant == "causal":
        tri = nc.dram_tensor("tri", [P, P], CDT, kind="ExternalInput")
    elif variant == "general":
        # mask.T pre-scaled by sqrt(HD) on host so exp's scale recovers it
        maskT = nc.dram_tensor("maskT", [S, S], CDT, kind="ExternalInput")
    y = nc.dram_tensor("y", [S, D], F32, kind="ExternalOutput")
    d_qrot = d_krot = d_vaug = d_attnT = None
    if dump:
        d_qrot = nc.dram_tensor("d_qrot", [P, HG, S], CDT, kind="ExternalOutput")
        d_krot = nc.dram_tensor("d_krot", [P, HG, S], CDT, kind="ExternalOutput")
        d_vaug = nc.dram_tensor("d_vaug", [P, NST, HG, HD + 2], CDT, kind="ExternalOutput")
        d_attnT = nc.dram_tensor("d_attnT", [P, HG, S], CDT, kind="ExternalOutput")

    with tile.TileContext(nc) as tc:
        with (
            tc.tile_pool(name="const", bufs=1) as constp,
            tc.tile_pool(name="big", bufs=1) as bigp,
            # one PSUM pool shared by every phase: no pool-scoping barriers,
            # so attention matmuls can start while the v-pass drains.
            tc.tile_pool(name="ps", bufs=6, space="PSUM") as pspool,
            tc.tile_pool(name="psaux", bufs=2, space="PSUM") as psaux,
        ):
            qrot = bigp.tile([P, HG, S], CDT, tag="qrot")
            krot = bigp.tile([P, HG, S], CDT, tag="krot")
            vaug = bigp.tile([P, NST, HG, HD + 2], CDT, tag="vaug")

            # ---------------- projections + RoPE ----------------
            # Weights are resident (loaded once; the first chunks are split
            # across partitions+engines so the first matmul starts ~3us in);
            # x panels stream per s-block and are reused by all three
            # projections.
            with (
                tc.tile_pool(name="wres", bufs=1) as wres,
                tc.tile_pool(name="xpool", bufs=2) as xpool,
                tc.tile_pool(name="rope", bufs=3) as ropep,
            ):
                wq_r = wres.tile([P, NKC, DG], CDT, tag="wq")
                wk_r = wres.tile([P, NKC, DG], CDT, tag="wk")
                wv_r = wres.tile([P, NKC, DG], CDT, tag="wv")
                xt0 = xpool.tile([P, NKC, SBLK], CDT, tag="xt", name="xp0")
                ENG = [nc.sync, nc.scalar, nc.gpsimd, nc.sync]
                # chunk 0 of wq and x-panel 0: 32-partition slices on four
                # engines each -> ~2us to first matmul
                for i, e in enumerate(ENG):
                    psl = slice(i * 32, (i + 1) * 32)
                    e.dma_start(wq_r[psl, 0, :], wq[psl, 0, :])
                for i, e in enumerate(ENG):
                    psl = slice(i * 32, (i + 1) * 32)
                    e.dma_start(xt0[psl, 0, :], xT[psl, 0, 0, :])
                # chunks 1-3 split in partition halves on rotating engines
                for c in range(1, 4):
                    for i2 in range(2):
                        psl = slice(i2 * 64, (i2 + 1) * 64)
                        ENG[(2 * c + i2) % 3].dma_start(
                            wq_r[psl, c, :], wq[psl, c, :]
                        )
                        ENG[(2 * c + i2 + 1) % 3].dma_start(
                            xt0[psl, c, :], xT[psl, 0, c, :]
                        )
                tid = constp.tile([P, P], CDT)
                nc.scalar.dma_start(tid[:], ident[:])
                ttri = None
                if variant == "causal":
                    ttri = constp.tile([P, P], CDT)
                    nc.scalar.dma_start(ttri[:], tri[:])
                tcos = constp.tile([HD, S], CDT)
                nc.scalar.dma_start(tcos[:], cosT[:])
                tsin = constp.tile([HD, S], CDT)
                nc.scalar.dma_start(tsin[:], sinT[:])
                # remaining chunks as two 6-chunk DMAs each: per-partition
                # rows are contiguous in the host layout, so these get 12KB
                # descriptors (fast) instead of per-chunk 1KB ones
                nc.sync.dma_start(wq_r[:, 4:10, :], wq[:, 4:10, :])
                nc.gpsimd.dma_start(xt0[:, 4:10, :], xT[:, 0, 4:10, :])
                nc.sync.dma_start(wq_r[:, 10:16, :], wq[:, 10:16, :])
                nc.gpsimd.dma_start(xt0[:, 10:16, :], xT[:, 0, 10:16, :])
                nc.sync.dma_start(wk_r[:], wk[:])
                nc.sync.dma_start(wv_r[:], wv[:])
                wmap = {"q": wq_r, "k": wk_r, "v": wv_r}

                ones128 = constp.tile([P, P], CDT)
                nc.vector.memset(ones128, 1.0)

                for sb in range(NSB):
                    if sb == 0:
                        xt = xt0
                    else:
                        xt = xpool.tile([P, NKC, SBLK], CDT, tag="xt", name=f"xp{sb}")
                        nc.gpsimd.dma_start(xt[:], xT[:, sb, :, :])
                    for proj in ("q", "k", "v"):
                        w_r = wmap[proj]
                        nun = SBLK // P if proj == "v" else HG
                        pss = [
                            pspool.tile(
                                [P, SBLK if proj != "v" else DG],
                                F32,
                                tag="ps",
                                name=f"ps_{proj}_{sb}_{u}",
                            )
                            for u in range(nun)
                        ]
                        for kc in range(NKC):
                            if proj in ("q", "k"):
                                for dt in range(HG):
                                    nc.tensor.matmul(
                                        pss[dt][:],
                                        w_r[:, kc, dt * HD : (dt + 1) * HD],
                                        xt[:, kc, :],
                                        start=(kc == 0),
                                        stop=(kc == NKC - 1),
                                    )
                            else:
                                for st in range(SBLK // P):
                                    nc.tensor.matmul(
                                        pss[st][:],
                                        xt[:, kc, st * P : (st + 1) * P],
                                        w_r[:, kc, :],
                                        start=(kc == 0),
                                        stop=(kc == NKC - 1),
                                    )
                        if proj in ("q", "k"):
                            dstbuf = qrot if proj == "q" else krot
                            ssl = slice(sb * SBLK, (sb + 1) * SBLK)
                            # drain all four PSUM banks first (fast copies),
                            # then run the RoPE chains from SBUF
                            qsbs = []
                            for dt in range(HG):
                                qsb = ropep.tile(
                                    [P, SBLK], CDT, tag=f"qsb{dt}", name="qsb"
                                )
                                nc.vector.tensor_copy(qsb[:], pss[dt][:])
                                qsbs.append(qsb)
                            for dt in range(HG):
                                qsb = qsbs[dt]
                                tsw = ropep.tile([P, SBLK], CDT, tag="tsw", name="tsw")
                                nc.vector.stream_shuffle(tsw[:], qsb[:], SWAP32)
                                t1 = ropep.tile([P, SBLK], CDT, tag="t1", name="t1")
                                nc.vector.tensor_mul(t1[:], qsb[:], tcos[:, ssl])
                                t2 = ropep.tile([P, SBLK], CDT, tag="t2", name="t2")
                                nc.vector.tensor_mul(t2[:], tsw[:], tsin[:, ssl])
                                nc.vector.tensor_add(
                                    dstbuf[:, dt, ssl], t1[:], t2[:]
                                )
                        else:
                            for st in range(SBLK // P):
                                st_g = sb * (SBLK // P) + st
                                nc.vector.tensor_copy(
                                    vaug[:, st_g, :, 0:HD],
                                    pss[st][:].rearrange("p (h d) -> p h d", d=HD),
                                )

            if dump:
                nc.sync.dma_start(d_qrot.ap(), qrot[:])
                nc.sync.dma_start(d_krot.ap(), krot[:])
                nc.sync.dma_start(d_vaug.ap(), vaug[:])

            # ---------------- attention (+ wo weights prefetch) ----------------
            with (
                tc.tile_pool(name="attn_out", bufs=1) as atp,
                tc.tile_pool(name="wopool", bufs=1) as wopool,
            ):
              attnT = atp.tile([P, HG, S], CDT, tag="attnT")
              wo_sb = wopool.tile([P, HG, D], CDT, tag="wo")
              nc.sync.dma_start(wo_sb[:], wo[:])
              with (
                tc.tile_pool(name="mask", bufs=2) as maskp,
                tc.tile_pool(name="expp", bufs=6) as epool,
                tc.tile_pool(name="esum", bufs=2) as esump,
                tc.tile_pool(name="normp", bufs=2) as npool,
                tc.tile_pool(name="outp", bufs=3) as outp,
              ):
                # out-projection work items (st, eb) deferred from the
                # previous q-block; emitted between score/PV matmuls so the
                # PE stays busy while ACT computes the exps.
                pending: list[tuple[int, int]] = []
                dma_eng = [nc.sync, nc.scalar, nc.gpsimd]
                dma_ctr = [0]

                def emit_outproj(n: int):
                    for _ in range(n):
                        if not pending:
                            return
                        st, eb = pending.pop(0)
                        ps_o = psaux.tile([P, EB], F32, tag="tr", name=f"o{st}_{eb}")
                        for dc in range(HG):
                            nc.tensor.matmul(
                                ps_o[:],
                                attnT[:, dc, st * P : (st + 1) * P],
                                wo_sb[:, dc, eb * EB : (eb + 1) * EB],
                                start=(dc == 0),
                                stop=(dc == HG - 1),
                            )
                        out_t = outp.tile([P, EB], F32, tag="outsb", name="outsb")
                        nc.vector.tensor_copy(out_t[:], ps_o[:])
                        eng = dma_eng[dma_ctr[0] % 3]
                        dma_ctr[0] += 1
                        eng.dma_start(
                            y[st * P : (st + 1) * P, eb * EB : (eb + 1) * EB],
                            out_t[:],
                        )

                # deferred per-head normalization: the denominator matmul +
                # reciprocal + attnT drain for head h are emitted early in
                # head h+1's score loop, hiding the DVE esum-chain latency.
                def emit_den(state):
                    att_ps, esum, s_qb, s_h = state
                    esum_bf = npool.tile([P, QB], CDT, tag="ebf", name="ebf")
                    nc.vector.tensor_copy(esum_bf[:], esum[:])
                    # sum across partitions AND broadcast, in one all-ones
                    # bf16 matmul (fp32r would downclock the whole PE)
                    den_ps = psaux.tile([P, QB], F32, tag="tr", name="den")
                    nc.tensor.matmul(
                        den_ps[:], ones128[:], esum_bf[:], start=True, stop=True
                    )
                    rec = npool.tile([P, QB], F32, tag="rec", name="rec")
                    nc.vector.reciprocal_approx_fast(rec[:], den_ps[:])
                    nc.vector.tensor_mul(
                        attnT[:, s_h, s_qb * QB : (s_qb + 1) * QB],
                        att_ps[:],
                        rec[:],
                    )

                prev_state = None
                for qb in range(NQB):
                    mts = None
                    if variant == "general":
                        mts = maskp.tile([P, NST, QB], CDT, tag="mt", name="mt")
                        nc.sync.dma_start(
                            mts[:],
                            maskT[:, qb * QB : (qb + 1) * QB].rearrange(
                                "(kt p) q -> p kt q", p=P
                            ),
                        )
                    nkt = NQT * (qb + 1) if variant == "causal" else NST
                    for h in range(HG):
                        att_ps = pspool.tile(
                            [P, QB], F32, tag="ps", name=f"att_{qb}_{h}"
                        )
                        esum = esump.tile([P, QB], F32, tag="esum", name="esum")
                        texp0 = None
                        den_at = 0 if h == 0 else 1
                        for kt in range(nkt):
                            j = kt - NQT * qb  # diag index (causal)
                            diag = variant == "causal" and j >= 0
                            ps_s = pspool.tile([P, QB], F32, tag="ps", name="scores")
                            if diag:
                                nc.tensor.matmul(
                                    ps_s[:, j * P : QB],
                                    krot[:, h, kt * P : (kt + 1) * P],
                                    qrot[:, h, qb * QB + j * P : (qb + 1) * QB],
                                    start=True,
                                    stop=False,
                                )
                                nc.tensor.matmul(
                                    ps_s[:, j * P : (j + 1) * P],
                                    tid[:],
                                    ttri[:],
                                    start=False,
                                    stop=True,
                                )
                                valid = slice(j * P, QB)
                            else:
                                last = variant != "general"
                                nc.tensor.matmul(
                                    ps_s[:],
                                    krot[:, h, kt * P : (kt + 1) * P],
                                    qrot[:, h, qb * QB : (qb + 1) * QB],
                                    start=True,
                                    stop=last,
                                )
                                if variant == "general":
                                    nc.tensor.matmul(
                                        ps_s[:],
                                        tid[:],
                                        mts[:, kt, :],
                                        start=False,
                                        stop=True,
                                    )
                                valid = slice(0, QB)
                            texp = epool.tile([P, QB], CDT, tag="exp", name="exp")
                            nc.scalar.activation(
                                texp[:, valid], ps_s[:, valid], EXP, scale=SCALE
                            )
                            # transposed PV accumulate: attnT_ps[d, q]
                            nc.tensor.matmul(
                                att_ps[:, valid],
                                vaug[:, kt, h, 0:HD],
                                texp[:, valid],
                                start=(kt == 0),
                                stop=(kt == nkt - 1),
                                skip_group_check=(variant == "causal"),
                            )
                            # denominator partials on DVE (esum starts at the
                            # kt==1 pair so the chain is one op shorter)
                            if kt == 0:
                                texp0 = texp
                            elif kt == 1 and not diag:
                                nc.vector.tensor_add(
                                    esum[:], texp0[:], texp[:]
                                )
                            else:
                                if kt == 1:
                                    # qb==0: kt1 is already diagonal/partial
                                    nc.vector.tensor_copy(esum[:], texp0[:])
                                nc.vector.tensor_add(
                                    esum[:, valid],
                                    esum[:, valid],
                                    texp[:, valid],
                                )
                            if kt == den_at and prev_state is not None:
                                emit_den(prev_state)
                                prev_state = None
                            # spread the deferred out-proj fill work evenly
                            # over this q-block's score/PV slots (16 items
                            # from qb-1 across 16*(qb+1) kt slots)
                            if kt % (qb + 1) == qb:
                                emit_outproj(1)
                        prev_state = (att_ps, esum, qb, h)
                    pending.extend(
                        (st, eb)
                        for st in range(qb * NQT, qb * NQT + NQT)
                        for eb in range(NEB)
                    )
                # final head's normalization + the last q-block's out-proj
                emit_den(prev_state)
                emit_outproj(len(pending))

              if dump:
                  nc.sync.dma_start(d_attnT.ap(), attnT[:])

    nc.compile()
    return nc


_PROGRAM_CACHE: dict[str, object] = {}
_last_in_maps = None


def _get_program(variant: str):
    key = f"{variant}:{COMPUTE_DTYPE}"
    if key not in _PROGRAM_CACHE:
        _PROGRAM_CACHE[key] = build_program(variant)
    return _PROGRAM_CACHE[key]


def _detect_variant(mask: np.ndarray) -> str:
    if not np.any(mask):
        return "none"
    causal = np.triu(np.full((S, S), NEG, dtype=np.float32), 1)
    if np.array_equal(mask, causal):
        return "causal"
    return "general"


def _np_cdt():
    if COMPUTE_DTYPE == "bfloat16":
        import ml_dtypes

        return ml_dtypes.bfloat16
    return np.float32


def make_in_maps(x, wq, wk, wv, wo, cos, sin, mask, variant):
    npdt = _np_cdt()
    cosT = np.repeat(cos.T, 2, axis=0)  # [HD, S]
    sinT = np.repeat(sin.T, 2, axis=0)
    sinT = sinT.copy()
    sinT[0::2, :] *= -1.0  # row 2i holds -sin, row 2i+1 holds +sin
    shared = {
        "cosT": np.ascontiguousarray(cosT).astype(npdt),
        "sinT": np.ascontiguousarray(sinT).astype(npdt),
        "ident": np.eye(P, dtype=np.float32).astype(npdt),
    }
    if variant == "causal":
        # scoresT layout is [kp, q]: masked where kp > q -> strict lower triangle
        shared["tri"] = np.tril(np.full((P, P), NEG, dtype=np.float32), -1).astype(npdt)
    elif variant == "general":
        shared["maskT"] = np.ascontiguousarray(mask.T * math.sqrt(HD)).astype(npdt)

    # xT chunked to [P, NSB, NKC, SBLK]: [p, sb, kc, s'] = x[b, sb*SBLK+s', kc*P+p]
    xTs = [
        np.ascontiguousarray(
            x[b].T.reshape(NKC, P, NSB, SBLK).transpose(1, 2, 0, 3)
        ).astype(npdt)
        for b in range(B)
    ]
    in_maps = []
    for core in range(NCORES):
        b, g = divmod(core, NCORES // B)
        sl = slice(g * DG, (g + 1) * DG)
        in_maps.append(
            {
                "xT": xTs[b],
                "wq": np.ascontiguousarray(
                    wq[:, sl].reshape(NKC, P, DG).transpose(1, 0, 2)
                ).astype(npdt),
                "wk": np.ascontiguousarray(
                    wk[:, sl].reshape(NKC, P, DG).transpose(1, 0, 2)
                ).astype(npdt),
                "wv": np.ascontiguousarray(
                    wv[:, sl].reshape(NKC, P, DG).transpose(1, 0, 2)
                ).astype(npdt),
                "wo": np.ascontiguousarray(
                    wo[sl, :].reshape(HG, P, D).transpose(1, 0, 2)
                ).astype(npdt),
                **shared,
            }
        )
    return in_maps


def kernel(x, wq, wk, wv, wo, cos, sin, mask):
    x = np.asarray(x, dtype=np.float32)
    wq = np.asarray(wq, dtype=np.float32)
    wk = np.asarray(wk, dtype=np.float32)
    wv = np.asarray(wv, dtype=np.float32)
    wo = np.asarray(wo, dtype=np.float32)
    cos = np.asarray(cos, dtype=np.float32)
    sin = np.asarray(sin, dtype=np.float32)
    mask = np.asarray(mask, dtype=np.float32)

    variant = _detect_variant(mask)
    nc = _get_program(variant)
    in_maps = make_in_maps(x, wq, wk, wv, wo, cos, sin, mask, variant)

    global _last_in_maps
    _last_in_maps = in_maps

    res = run_bass_kernel_spmd(nc, in_maps, core_ids=list(range(NCORES)))

    out = np.empty((B, S, D), dtype=np.float32)
    gpb = NCORES // B
    for b in range(B):
        acc = np.zeros((S, D), dtype=np.float64)
        for g in range(gpb):
            acc += res.results[b * gpb + g]["y"].astype(np.float64)
        out[b] = acc.astype(np.float32)
    return out


# revision 23
# speedup vs baseline: 1.1869x; 1.1735x over previous
"""Fused multi-head attention (QKV proj + RoPE + causal softmax + out proj)
for Trainium2, sharded over 8 NeuronCores.

Sharding: data-parallel over batch (B=2) x tensor-parallel over heads
(16 heads -> 4 per core).  Each core computes, for its (batch, head-group):
  qT/kT = wq/wk^T-projections in [d, s] layout (CDT matmuls, fp32 PSUM)
  RoPE applied on-chip (DVE pair-swap via stream_shuffle + mul/add)
  scoresT[kp, q] = krot^T.T @ qrot (one K=128 matmul per tile)
  causal masking via a PE-accumulated triangular constant on diagonal tiles
  expT = exp(scale * scoresT) on ACT
  transposed PV: attnT[d, q] += vaug[k, d].T @ expT[k, q]  (512-wide matmuls)
  denominator: DVE-accumulated exp sums -> one all-ones f32r matmul that both
  sums across partitions and broadcasts the result; reciprocal folded into
  the attnT drain (PSUM * rec -> SBUF)
  partial output y_g = attnT.T @ wo_rows  (summed over head-groups on host),
  software-pipelined into the next q-block's score/PV loops so the PE never
  waits on ACT exp latency.

Weights are loaded once into resident SBUF tiles (host pre-arranges them
into the on-chip [P, chunk, free] layout so DMA descriptors are large);
x arrives as per-s-block panels reused by all three projections.

Inputs arrive full-size; host slices/transposes, feeds 8 SPMD cores, and
sums the 4 head-group partials per batch at the end.
"""

import math

import numpy as np

import concourse.bacc as bacc
import concourse.mybir as mybir
from concourse import tile
from concourse.bass_utils import run_bass_kernel_spmd

B, S, D, H = 2, 2048, 2048, 16
NCORES = 8
HG = 4  # heads per core
HD = D // H  # 128
DG = HG * HD  # 512 = per-core slice of D
P = 128
NKC = D // P  # 16 contraction chunks
SBLK = 512  # s-block width in projection passes
NSB = S // SBLK
NST = S // P  # 16 s-tiles of 128
QB = 512  # q-block width in attention
NQB = S // QB
NQT = QB // P  # q-subtiles per block
EB = 512  # e-block width in out-projection
NEB = D // EB

F32 = mybir.dt.float32
F32R = mybir.dt.float32r
EXP = mybir.ActivationFunctionType.Exp
SCALE = 1.0 / math.sqrt(HD)
SWAP32 = [i ^ 1 for i in range(32)]
NEG = -1.0e9

COMPUTE_DTYPE = "bfloat16"


def build_program(variant: str, dump: bool = False, cdt_name: str | None = None):
    """variant: 'causal' | 'none' | 'general'"""
    CDT = getattr(mybir.dt, cdt_name or COMPUTE_DTYPE)
    nc = bacc.Bacc("TRN2", target_bir_lowering=False, debug=False)
    # host pre-arranged layouts (see make_in_maps):
    #   xT  [P, NSB, NKC, SBLK]   x^T chunked: [p, sb, kc, s'] = x[sb*512+s', kc*128+p]
    #   wq/wk/wv [P, NKC, DG]     [p, kc, d'] = w[kc*128+p, d']
    #   wo  [P, HG, D]            [p, dc, e]  = wo[dc*128+p, e]
    xT = nc.dram_tensor("xT", [P, NSB, NKC, SBLK], CDT, kind="ExternalInput")
    wq = nc.dram_tensor("wq", [P, NKC, DG], CDT, kind="ExternalInput")
    wk = nc.dram_tensor("wk", [P, NKC, DG], CDT, kind="ExternalInput")
    wv = nc.dram_tensor("wv", [P, NKC, DG], CDT, kind="ExternalInput")
    wo = nc.dram_tensor("wo", [P, HG, D], CDT, kind="ExternalInput")
    cosT = nc.dram_tensor("cosT", [HD, S], CDT, kind="ExternalInput")
    sinT = nc.dram_tensor("sinT", [HD, S], CDT, kind="ExternalInput")
    ident = nc.dram_tensor("ident", [P, P], CDT, kind="ExternalInput")
    tri = None
    maskT = None
    if variant == "causal":
        tri = nc.dram_tensor("tri", [P, P], CDT, kind="ExternalInput")
    elif variant == "general":
        # mask.T pre-scaled by sqrt(HD) on host so exp's scale recovers it
        maskT = nc.dram_tensor("maskT", [S, S], CDT, kind="ExternalInput")
    y = nc.dram_tensor("y", [S, D], F32, kind="ExternalOutput")
    d_qrot = d_krot = d_vaug = d_attnT = None
    if dump:
        d_qrot = nc.dram_tensor("d_qrot", [P, HG, S], CDT, kind="ExternalOutput")
        d_krot = nc.dram_tensor("d_krot", [P, HG, S], CDT, kind="ExternalOutput")
        d_vaug = nc.dram_tensor("d_vaug", [P, NST, HG, HD + 2], CDT, kind="ExternalOutput")
        d_attnT = nc.dram_tensor("d_attnT", [P, HG, S], CDT, kind="ExternalOutput")

    with tile.TileContext(nc) as tc:
        with (
            tc.tile_pool(name="const", bufs=1) as constp,
            tc.tile_pool(name="big", bufs=1) as bigp,
            # one PSUM pool shared by every phase: no pool-scoping barriers,
            # so attention matmuls can start while the v-pass drains.
            tc.tile_pool(name="ps", bufs=6, space="PSUM") as pspool,
            tc.tile_pool(name="psaux", bufs=2, space="PSUM") as psaux,
        ):
            qrot = bigp.tile([P, HG, S], CDT, tag="qrot")
            krot = bigp.tile([P, HG, S], CDT, tag="krot")
            vaug = bigp.tile([P, NST, HG, HD + 2], CDT, tag="vaug")

            # ---------------- projections + RoPE ----------------
            # Weights are resident (loaded once; the first chunks are split
            # across partitions+engines so the first matmul starts ~3us in);
            # x panels stream per s-block and are reused by all three
            # projections.
            with (
                tc.tile_pool(name="wres", bufs=1) as wres,
                tc.tile_pool(name="xpool", bufs=2) as xpool,
                tc.tile_pool(name="rope", bufs=3) as ropep,
            ):
                wq_r = wres.tile([P, NKC, DG], CDT, tag="wq")
                wk_r = wres.tile([P, NKC, DG], CDT, tag="wk")
                wv_r = wres.tile([P, NKC, DG], CDT, tag="wv")
                xt0 = xpool.tile([P, NKC, SBLK], CDT, tag="xt", name="xp0")
                ENG = [nc.sync, nc.scalar, nc.gpsimd, nc.sync]
                # chunk 0 of wq and x-panel 0: 32-partition slices on four
                # engines each -> ~2us to first matmul
                for i, e in enumerate(ENG):
                    psl = slice(i * 32, (i + 1) * 32)
                    e.dma_start(wq_r[psl, 0, :], wq[psl, 0, :])
                for i, e in enumerate(ENG):
                    psl = slice(i * 32, (i + 1) * 32)
                    e.dma_start(xt0[psl, 0, :], xT[psl, 0, 0, :])
                # chunks 1-3 split in partition halves on rotating engines
                for c in range(1, 4):
                    for i2 in range(2):
                        psl = slice(i2 * 64, (i2 + 1) * 64)
                        ENG[(2 * c + i2) % 3].dma_start(
                            wq_r[psl, c, :], wq[psl, c, :]
                        )
                        ENG[(2 * c + i2 + 1) % 3].dma_start(
                            xt0[psl, c, :], xT[psl, 0, c, :]
                        )
                tid = constp.tile([P, P], CDT)
                nc.scalar.dma_start(tid[:], ident[:])
                ttri = None
                if variant == "causal":
                    ttri = constp.tile([P, P], CDT)
                    nc.scalar.dma_start(ttri[:], tri[:])
                tcos = constp.tile([HD, S], CDT)
                nc.scalar.dma_start(tcos[:], cosT[:])
                tsin = constp.tile([HD, S], CDT)
                nc.scalar.dma_start(tsin[:], sinT[:])
                # remaining chunks as two 6-chunk DMAs each: per-partition
                # rows are contiguous in the host layout, so these get 12KB
                # descriptors (fast) instead of per-chunk 1KB ones
                nc.sync.dma_start(wq_r[:, 4:10, :], wq[:, 4:10, :])
                nc.gpsimd.dma_start(xt0[:, 4:10, :], xT[:, 0, 4:10, :])
                nc.sync.dma_start(wq_r[:, 10:16, :], wq[:, 10:16, :])
                nc.gpsimd.dma_start(xt0[:, 10:16, :], xT[:, 0, 10:16, :])
                nc.sync.dma_start(wk_r[:], wk[:])
                nc.sync.dma_start(wv_r[:], wv[:])
                wmap = {"q": wq_r, "k": wk_r, "v": wv_r}

                ones128 = constp.tile([P, P], CDT)
                nc.vector.memset(ones128, 1.0)

                for sb in range(NSB):
                    if sb == 0:
                        xt = xt0
                    else:
                        xt = xpool.tile([P, NKC, SBLK], CDT, tag="xt", name=f"xp{sb}")
                        nc.gpsimd.dma_start(xt[:], xT[:, sb, :, :])
                    for proj in ("q", "k", "v"):
                        w_r = wmap[proj]
                        nun = SBLK // P if proj == "v" else HG
                        pss = [
                            pspool.tile(
                                [P, SBLK if proj != "v" else DG],
                                F32,
                                tag="ps",
                                name=f"ps_{proj}_{sb}_{u}",
                            )
                            for u in range(nun)
                        ]
                        for kc in range(NKC):
                            if proj in ("q", "k"):
                                for dt in range(HG):
                                    nc.tensor.matmul(
                                        pss[dt][:],
                                        w_r[:, kc, dt * HD : (dt + 1) * HD],
                                        xt[:, kc, :],
                                        start=(kc == 0),
                                        stop=(kc == NKC - 1),
                                    )
                            else:
                                for st in range(SBLK // P):
                                    nc.tensor.matmul(
                                        pss[st][:],
                                        xt[:, kc, st * P : (st + 1) * P],
                                        w_r[:, kc, :],
                                        start=(kc == 0),
                                        stop=(kc == NKC - 1),
                                    )
                        if proj in ("q", "k"):
                            dstbuf = qrot if proj == "q" else krot
                            ssl = slice(sb * SBLK, (sb + 1) * SBLK)
                            # drain all four PSUM banks first (fast copies),
                            # then run the RoPE chains from SBUF
                            qsbs = []
                            for dt in range(HG):
                                qsb = ropep.tile(
                                    [P, SBLK], CDT, tag=f"qsb{dt}", name="qsb"
                                )
                                nc.vector.tensor_copy(qsb[:], pss[dt][:])
                                qsbs.append(qsb)
                            for dt in range(HG):
                                qsb = qsbs[dt]
                                tsw = ropep.tile([P, SBLK], CDT, tag="tsw", name="tsw")
                                nc.vector.stream_shuffle(tsw[:], qsb[:], SWAP32)
                                t1 = ropep.tile([P, SBLK], CDT, tag="t1", name="t1")
                                nc.vector.tensor_mul(t1[:], qsb[:], tcos[:, ssl])
                                t2 = ropep.tile([P, SBLK], CDT, tag="t2", name="t2")
                                nc.vector.tensor_mul(t2[:], tsw[:], tsin[:, ssl])
                                nc.vector.tensor_add(
                                    dstbuf[:, dt, ssl], t1[:], t2[:]
                                )
                        else:
                            for st in range(SBLK // P):
                                st_g = sb * (SBLK // P) + st
                                nc.vector.tensor_copy(
                                    vaug[:, st_g, :, 0:HD],
                                    pss[st][:].rearrange("p (h d) -> p h d", d=HD),
                                )

            if dump:
                nc.sync.dma_start(d_qrot.ap(), qrot[:])
                nc.sync.dma_start(d_krot.ap(), krot[:])
                nc.sync.dma_start(d_vaug.ap(), vaug[:])

            # ---------------- attention (+ wo weights prefetch) ----------------
            with (
                tc.tile_pool(name="attn_out", bufs=1) as atp,
                tc.tile_pool(name="wopool", bufs=1) as wopool,
            ):
              attnT = atp.tile([P, HG, S], CDT, tag="attnT")
              wo_sb = wopool.tile([P, HG, D], CDT, tag="wo")
              nc.sync.dma_start(wo_sb[:], wo[:])
              with (
                tc.tile_pool(name="mask", bufs=2) as maskp,
                tc.tile_pool(name="expp", bufs=6) as epool,
                tc.tile_pool(name="esum", bufs=2) as esump,
                tc.tile_pool(name="normp", bufs=2) as npool,
                tc.tile_pool(name="outp", bufs=3) as outp,
              ):
                # out-projection work items (st, eb) deferred from the
                # previous q-block; emitted between score/PV matmuls so the
                # PE stays busy while ACT computes the exps.
                pending: list[tuple[int, int]] = []
                dma_eng = [nc.sync, nc.scalar, nc.gpsimd]
                dma_ctr = [0]

                out_rows: dict[int, object] = {}

                def emit_outproj(n: int):
                    for _ in range(n):
                        if not pending:
                            return
                        st, eb = pending.pop(0)
                        ps_o = psaux.tile([P, EB], F32, tag="tr", name=f"o{st}_{eb}")
                        for dc in range(HG):
                            nc.tensor.matmul(
                                ps_o[:],
                                attnT[:, dc, st * P : (st + 1) * P],
                                wo_sb[:, dc, eb * EB : (eb + 1) * EB],
                                start=(dc == 0),
                                stop=(dc == HG - 1),
                            )
                        # batch the four eb slices of an s-tile into one row
                        # buffer and write y with a single 8KB-descriptor DMA
                        if st not in out_rows:
                            out_rows[st] = outp.tile(
                                [P, D], F32, tag="outsb", name=f"or{st}"
                            )
                        out_row = out_rows[st]
                        nc.vector.tensor_copy(
                            out_row[:, eb * EB : (eb + 1) * EB], ps_o[:]
                        )
                        if eb == NEB - 1:
                            del out_rows[st]
                            eng = dma_eng[dma_ctr[0] % 3]
                            dma_ctr[0] += 1
                            eng.dma_start(y[st * P : (st + 1) * P, :], out_row[:])

                # deferred per-head normalization: the denominator matmul +
                # reciprocal + attnT drain for head h are emitted early in
                # head h+1's score loop, hiding the DVE esum-chain latency.
                def emit_den(state):
                    att_ps, esum, s_qb, s_h = state
                    esum_bf = npool.tile([P, QB], CDT, tag="ebf", name="ebf")
                    nc.vector.tensor_copy(esum_bf[:], esum[:])
                    # sum across partitions AND broadcast, in one all-ones
                    # bf16 matmul (fp32r would downclock the whole PE)
                    den_ps = psaux.tile([P, QB], F32, tag="tr", name="den")
                    nc.tensor.matmul(
                        den_ps[:], ones128[:], esum_bf[:], start=True, stop=True
                    )
                    rec = npool.tile([P, QB], F32, tag="rec", name="rec")
                    nc.vector.reciprocal_approx_fast(rec[:], den_ps[:])
                    nc.vector.tensor_mul(
                        attnT[:, s_h, s_qb * QB : (s_qb + 1) * QB],
                        att_ps[:],
                        rec[:],
                    )

                prev_state = None
                for qb in range(NQB):
                    mts = None
                    if variant == "general":
                        mts = maskp.tile([P, NST, QB], CDT, tag="mt", name="mt")
                        nc.sync.dma_start(
                            mts[:],
                            maskT[:, qb * QB : (qb + 1) * QB].rearrange(
                                "(kt p) q -> p kt q", p=P
                            ),
                        )
                    nkt = NQT * (qb + 1) if variant == "causal" else NST
                    for h in range(HG):
                        att_ps = pspool.tile(
                            [P, QB], F32, tag="ps", name=f"att_{qb}_{h}"
                        )
                        esum = esump.tile([P, QB], F32, tag="esum", name="esum")
                        texp0 = None
                        den_at = 0 if h == 0 else 1
                        for kt in range(nkt):
                            j = kt - NQT * qb  # diag index (causal)
                            diag = variant == "causal" and j >= 0
                            ps_s = pspool.tile([P, QB], F32, tag="ps", name="scores")
                            if diag:
                                nc.tensor.matmul(
                                    ps_s[:, j * P : QB],
                                    krot[:, h, kt * P : (kt + 1) * P],
                                    qrot[:, h, qb * QB + j * P : (qb + 1) * QB],
                                    start=True,
                                    stop=False,
                                )
                                nc.tensor.matmul(
                                    ps_s[:, j * P : (j + 1) * P],
                                    tid[:],
                                    ttri[:],
                                    start=False,
                                    stop=True,
                                )
                                valid = slice(j * P, QB)
                            else:
                                last = variant != "general"
                                nc.tensor.matmul(
                                    ps_s[:],
                                    krot[:, h, kt * P : (kt + 1) * P],
                                    qrot[:, h, qb * QB : (qb + 1) * QB],
                                    start=True,
                                    stop=last,
                                )
                                if variant == "general":
                                    nc.tensor.matmul(
                                        ps_s[:],
                                        tid[:],
                                        mts[:, kt, :],
                                        start=False,
                                        stop=True,
                                    )
                                valid = slice(0, QB)
                            texp = epool.tile([P, QB], CDT, tag="exp", name="exp")
                            nc.scalar.activation(
                                texp[:, valid], ps_s[:, valid], EXP, scale=SCALE
                            )
                            # transposed PV accumulate: attnT_ps[d, q]
                            nc.tensor.matmul(
                                att_ps[:, valid],
                                vaug[:, kt, h, 0:HD],
                                texp[:, valid],
                                start=(kt == 0),
                                stop=(kt == nkt - 1),
                                skip_group_check=(variant == "causal"),
                            )
                            # denominator partials on DVE (esum starts at the
                            # kt==1 pair so the chain is one op shorter)
                            if kt == 0:
                                texp0 = texp
                            elif kt == 1 and not diag:
                                nc.vector.tensor_add(
                                    esum[:], texp0[:], texp[:]
                                )
                            else:
                                if kt == 1:
                                    # qb==0: kt1 is already diagonal/partial
                                    nc.vector.tensor_copy(esum[:], texp0[:])
                                nc.vector.tensor_add(
                                    esum[:, valid],
                                    esum[:, valid],
                                    texp[:, valid],
                                )
                            if kt == den_at and prev_state is not None:
                                emit_den(prev_state)
                                prev_state = None
                            # spread the deferred out-proj fill work evenly
                            # over this q-block's score/PV slots (16 items
                            # from qb-1 across 16*(qb+1) kt slots)
                            if kt % (qb + 1) == qb:
                                emit_outproj(1)
                        prev_state = (att_ps, esum, qb, h)
                    pending.extend(
                        (st, eb)
                        for st in range(qb * NQT, qb * NQT + NQT)
                        for eb in range(NEB)
                    )
                # final head's normalization + the last q-block's out-proj
                emit_den(prev_state)
                emit_outproj(len(pending))

              if dump:
                  nc.sync.dma_start(d_attnT.ap(), attnT[:])

    nc.compile()
    return nc


_PROGRAM_CACHE: dict[str, object] = {}
_last_in_maps = None


def _get_program(variant: str):
    key = f"{variant}:{COMPUTE_DTYPE}"
    if key not in _PROGRAM_CACHE:
        _PROGRAM_CACHE[key] = build_program(variant)
    return _PROGRAM_CACHE[key]


def _detect_variant(mask: np.ndarray) -> str:
    if not np.any(mask):
        return "none"
    causal = np.triu(np.full((S, S), NEG, dtype=np.float32), 1)
    if np.array_equal(mask, causal):
        return "causal"
    return "general"


def _np_cdt():
    if COMPUTE_DTYPE == "bfloat16":
        import ml_dtypes

        return ml_dtypes.bfloat16
    return np.float32


def make_in_maps(x, wq, wk, wv, wo, cos, sin, mask, variant):
    npdt = _np_cdt()
    cosT = np.repeat(cos.T, 2, axis=0)  # [HD, S]
    sinT = np.repeat(sin.T, 2, axis=0)
    sinT = sinT.copy()
    sinT[0::2, :] *= -1.0  # row 2i holds -sin, row 2i+1 holds +sin
    shared = {
        "cosT": np.ascontiguousarray(cosT).astype(npdt),
        "sinT": np.ascontiguousarray(sinT).astype(npdt),
        "ident": np.eye(P, dtype=np.float32).astype(npdt),
    }
    if variant == "causal":
        # scoresT layout is [kp, q]: masked where kp > q -> strict lower triangle
        shared["tri"] = np.tril(np.full((P, P), NEG, dtype=np.float32), -1).astype(npdt)
    elif variant == "general":
        shared["maskT"] = np.ascontiguousarray(mask.T * math.sqrt(HD)).astype(npdt)

    # xT chunked to [P, NSB, NKC, SBLK]: [p, sb, kc, s'] = x[b, sb*SBLK+s', kc*P+p]
    xTs = [
        np.ascontiguousarray(
            x[b].T.reshape(NKC, P, NSB, SBLK).transpose(1, 2, 0, 3)
        ).astype(npdt)
        for b in range(B)
    ]
    in_maps = []
    for core in range(NCORES):
        b, g = divmod(core, NCORES // B)
        sl = slice(g * DG, (g + 1) * DG)
        in_maps.append(
            {
                "xT": xTs[b],
                "wq": np.ascontiguousarray(
                    wq[:, sl].reshape(NKC, P, DG).transpose(1, 0, 2)
                ).astype(npdt),
                "wk": np.ascontiguousarray(
                    wk[:, sl].reshape(NKC, P, DG).transpose(1, 0, 2)
                ).astype(npdt),
                "wv": np.ascontiguousarray(
                    wv[:, sl].reshape(NKC, P, DG).transpose(1, 0, 2)
                ).astype(npdt),
                "wo": np.ascontiguousarray(
                    wo[sl, :].reshape(HG, P, D).transpose(1, 0, 2)
                ).astype(npdt),
                **shared,
            }
        )
    return in_maps


def kernel(x, wq, wk, wv, wo, cos, sin, mask):
    x = np.asarray(x, dtype=np.float32)
    wq = np.asarray(wq, dtype=np.float32)
    wk = np.asarray(wk, dtype=np.float32)
    wv = np.asarray(wv, dtype=np.float32)
    wo = np.asarray(wo, dtype=np.float32)
    cos = np.asarray(cos, dtype=np.float32)
    sin = np.asarray(sin, dtype=np.float32)
    mask = np.asarray(mask, dtype=np.float32)

    variant = _detect_variant(mask)
    nc = _get_program(variant)
    in_maps = make_in_maps(x, wq, wk, wv, wo, cos, sin, mask, variant)

    global _last_in_maps
    _last_in_maps = in_maps

    res = run_bass_kernel_spmd(nc, in_maps, core_ids=list(range(NCORES)))

    out = np.empty((B, S, D), dtype=np.float32)
    gpb = NCORES // B
    for b in range(B):
        acc = np.zeros((S, D), dtype=np.float64)
        for g in range(gpb):
            acc += res.results[b * gpb + g]["y"].astype(np.float64)
        out[b] = acc.astype(np.float32)
    return out
